# revision 1
# baseline (speedup 1.0000x reference)
"""Supervised-contrastive loss on 8 Trainium2 NeuronCores.

Math (reference):
    z = x / max(||x||, 1e-8)                  row-normalize
    sim = (z @ z.T) / TEMP                    [N, N]
    per-anchor: pos-mean over same-class (excl. self) and logsumexp over
    j != i, then per-class mean, then mean over classes.

Distribution: rows of z (anchors) are sharded 1024/core across 8 cores;
z is replicated.  Each core computes its [1024, 8192] slab of raw cosine
similarities (fp8-e4m3 inputs via DoubleRow matmuls, fp32 PSUM
accumulate) and reduces it on the fly to two tiny outputs:
    esp[i, g] = sum_{j in group g} exp(10 * sim[i, j])   (incl. diagonal)
    tm[i, c]  = sum_{j in class c} sim[i, j]             (incl. diagonal)
The exp row-sums ride on the ScalarE activation via accum_out (no extra
reduction pass).  The class-segment sums are folded into a small GEMM:
tm = A @ W.T where W[c] = sum of z rows of class c (precomputed on
host), so no masking is needed on device.  The diagonal sim[i,i] =
||z8[i]||^2 is reconstructed exactly on host and subtracted there.  The
final logsumexp/segment-mean arithmetic over 8192 anchors is negligible
host work.

Layout: all fp8 operands are host-packed for DoubleRow so that feature
d = kk*256 + i*128 + p lands on partition p, plane i of contraction tile
kk, giving 2-6KB-contiguous per-partition DMA descriptors.

Hardware pitfalls baked into this structure (each verified by a device
crash or a multi-us regression): DMAs only from nc.sync (scalar-HWDGE
and gpsimd-SWDGE both misbehave), one matmul accumulation group per
PSUM bank, full-128-partition DoubleRow outputs only (M=64 out crashes),
and one EXP per psum tile (slice-wise EXP serializes the PE).
"""

import numpy as np
import ml_dtypes

N = 8192          # anchors
D = 768           # feature dim
NOP = 64          # number of classes
CORES = 8
SLAB = N // CORES  # 1024 anchors per core
KT8 = D // 256     # 3 double-row contraction tiles
MT = SLAB // 128   # 8 anchor chunks of 128 (PSUM partition dim)
JW = 512           # matmul free width (one PSUM bank)
GW = 2048          # j-group width (one wide PSUM tile / DMA chunk)
NG = N // GW       # 4 groups
GJ = GW // JW      # 4 matmul slices per group
TEMP_INV = 10.0
EPS = 1e-8

FP8 = ml_dtypes.float8_e4m3

_CACHE = {}
LAST_RESULT = None  # BassKernelResults of the most recent run (for profiling)


def _build_nc():
    from concourse import bacc
    import concourse.mybir as mybir
    import concourse.tile as tile

    f8 = mybir.dt.float8e4
    f32 = mybir.dt.float32
    Exp = mybir.ActivationFunctionType.Exp
    DR = mybir.MatmulPerfMode.DoubleRow

    nc = bacc.Bacc(
        "TRN2", target_bir_lowering=False, debug=False, enable_asserts=False
    )
    z8 = nc.dram_tensor("z8", [128, KT8, NG, 2, GW], f8, kind="ExternalInput").ap()
    a8 = nc.dram_tensor("a8", [128, KT8, 2, SLAB], f8, kind="ExternalInput").ap()
    w8 = nc.dram_tensor("w8", [128, KT8, 2, NOP], f8, kind="ExternalInput").ap()
    pout = nc.dram_tensor("pout", [128, MT * NG + 2], f32, kind="ExternalOutput").ap()
    tm = nc.dram_tensor("tm", [128, MT, NOP], f32, kind="ExternalOutput").ap()

    with tile.TileContext(nc) as tc:
        with (
            tc.tile_pool(name="zin", bufs=KT8 * NG) as zin,
            tc.tile_pool(name="singles", bufs=1) as singles,
        ):
            # ---- input DMAs (small/early operands first) ----
            w8_sb = singles.tile([128, KT8, 2, NOP], f8)
            nc.sync.dma_start(out=w8_sb, in_=w8)
            a8_sb = singles.tile([128, KT8, 2, SLAB], f8)

            def dma_a8_half(half):
                if half > 0:
                    return
                nc.sync.dma_start(
                    out=a8_sb.rearrange("p a b c -> p (a b c)"),
                    in_=a8.rearrange("p a b c -> p (a b c)"),
                )

            z8_sb = {}

            def dma_z8_group(g):
                for kk in range(KT8):
                    z8_t = zin.tile([128, 2, GW], f8, name="z8_t", tag="z8_t")
                    nc.sync.dma_start(out=z8_t, in_=z8[:, kk, g, :, :])
                    z8_sb[(g, kk)] = z8_t

            dma_a8_half(0)
            dma_a8_half(1)
            for g in range(NG):
                dma_z8_group(g)

            pacc = singles.tile([128, MT * NG + 2], f32)
            # the last (g, m) iteration writes the two tail slots instead
            nc.vector.memset(pacc[:, MT * NG - 1:MT * NG], 0.0)
            tm_sb = singles.tile([128, MT, NOP], f32)

            ps_pool = tc.alloc_tile_pool(name="ps", bufs=2, space="PSUM")

            # ---- class-segment sums: tm[:, m, c] = A_m @ W.T ----
            for m in range(MT):
                pst = ps_pool.tile([128, NOP], f32, name="ps_t", tag="ps_t")
                for kk in range(KT8):
                    nc.tensor.matmul(
                        pst,
                        a8_sb[:, kk, :, m * 128:(m + 1) * 128],
                        w8_sb[:, kk, :, :],
                        start=(kk == 0),
                        stop=(kk == KT8 - 1),
                        perf_mode=DR,
                    )
                nc.vector.tensor_copy(tm_sb[:, m, :], pst)
            nc.sync.dma_start(out=tm, in_=tm_sb)

            # ---- main similarity slab (fp8 DoubleRow) + fused exp sums ----
            for g in range(NG):
                for m in range(MT):
                    last = (g == NG - 1) and (m == MT - 1)
                    if not last:
                        ps_t = ps_pool.tile([128, GW], f32, name="ps_t", tag="ps_t")
                        for kk in range(KT8):
                            lhsT = a8_sb[:, kk, :, m * 128:(m + 1) * 128]
                            for jj in range(GJ):
                                nc.tensor.matmul(
                                    ps_t[:, jj * JW:(jj + 1) * JW],
                                    lhsT,
                                    z8_sb[(g, kk)][:, :, jj * JW:(jj + 1) * JW],
                                    start=(kk == 0),
                                    stop=(kk == KT8 - 1),
                                    perf_mode=DR,
                                )
                        nc.scalar.activation(
                            out=ps_t,
                            in_=ps_t,
                            func=Exp,
                            scale=TEMP_INV,
                            accum_out=pacc[:, m * NG + g:m * NG + g + 1],
                        )
                    else:
                        # final iteration: two half-width tiles from the same
                        # slots, so the first EXP starts before the last MMs
                        # finish (separate tiles -> no intra-tile serialization)
                        for h in range(2):
                            ps_h = ps_pool.tile(
                                [128, GW // 2], f32, name="ps_t", tag="ps_t"
                            )
                            for kk in range(KT8):
                                lhsT = a8_sb[:, kk, :, m * 128:(m + 1) * 128]
                                for jj in range(2):
                                    j = h * 2 + jj
                                    nc.tensor.matmul(
                                        ps_h[:, jj * JW:(jj + 1) * JW],
                                        lhsT,
                                        z8_sb[(g, kk)][:, :, j * JW:(j + 1) * JW],
                                        start=(kk == 0),
                                        stop=(kk == KT8 - 1),
                                        perf_mode=DR,
                                    )
                            nc.scalar.activation(
                                out=ps_h,
                                in_=ps_h,
                                func=Exp,
                                scale=TEMP_INV,
                                accum_out=pacc[:, MT * NG + h:MT * NG + h + 1],
                            )
            ps_pool.release()

            nc.sync.dma_start(out=pout, in_=pacc)

    nc.compile()
    return nc


def _get_nc():
    if "nc" not in _CACHE:
        _CACHE["nc"] = _build_nc()
    return _CACHE["nc"]


def _pack_dr(mat_t):
    """[D, cols] -> [128, KT8, 2, cols] with d = kk*256 + i*128 + p."""
    d, cols = mat_t.shape
    return np.ascontiguousarray(
        mat_t.reshape(KT8, 2, 128, cols).transpose(2, 0, 1, 3)
    )


def kernel(x, op_ids, n_op):
    global LAST_RESULT
    from concourse.bass_utils import run_bass_kernel_spmd

    x = np.asarray(x, dtype=np.float32).reshape(-1, D)
    op_ids = np.asarray(op_ids).reshape(-1).astype(np.int64)
    n_op_i = int(np.asarray(n_op))

    # ---- host prep: normalize, quantize, class sums, diagonal ----
    norms = np.sqrt((x.astype(np.float64) ** 2).sum(axis=1))
    norms = np.maximum(norms, EPS).astype(np.float32)
    z = x / norms[:, None]

    z8 = z.astype(FP8)
    z8f = z8.astype(np.float32)

    onehot = np.zeros((N, NOP), np.float32)
    onehot[np.arange(N), op_ids] = 1.0
    W8 = (onehot.T @ z8f).astype(FP8)               # [NOP, D] fp8

    z8_packed = _pack_dr(np.ascontiguousarray(z8.T))          # [128,3,2,N]
    # [128, KT8, NG, 2, GW]: each (g, kk) chunk contiguous per partition
    z8_chunked = np.ascontiguousarray(
        z8_packed.reshape(128, KT8, 2, NG, GW).transpose(0, 1, 3, 2, 4)
    )
    w8_packed = _pack_dr(np.ascontiguousarray(W8.T.astype(FP8)))
    ssq = (z8f.astype(np.float64) ** 2).sum(axis=1)  # = sim[i, i]

    in_maps = [
        {
            "z8": z8_chunked,
            "a8": np.ascontiguousarray(z8_packed[:, :, :, c * SLAB:(c + 1) * SLAB]),
            "w8": w8_packed,
        }
        for c in range(CORES)
    ]

    nc = _get_nc()
    res = run_bass_kernel_spmd(nc, in_maps, core_ids=list(range(CORES)))
    LAST_RESULT = res

    # ---- host post: stitch slabs, subtract diagonal, finish loss ----
    es_slabs = []
    tm_slabs = []
    for c in range(CORES):
        pout_c = res.results[c]["pout"].astype(np.float64)  # [128, MT*NG+2]
        esp_c = pout_c[:, :MT * NG].reshape(128, MT, NG)
        es_c = esp_c.sum(axis=2)
        # last (g, m) iteration wrote its two half-sums to the extra slots
        es_c[:, MT - 1] = (
            esp_c[:, MT - 1, :NG - 1].sum(axis=1) + pout_c[:, MT * NG:].sum(axis=1)
        )
        es_slabs.append(es_c.T.reshape(SLAB))
        tm_slabs.append(
            res.results[c]["tm"].transpose(1, 0, 2).reshape(SLAB, NOP)
        )
    es_full = np.concatenate(es_slabs)
    tm_full = np.concatenate(tm_slabs).astype(np.float64)

    lse = np.log(es_full - np.exp(TEMP_INV * ssq))
    pos_sum = TEMP_INV * (tm_full[np.arange(N), op_ids] - ssq)
    counts = np.bincount(op_ids, minlength=n_op_i).astype(np.float64)
    pos_cnt = counts[op_ids] - 1.0

    loss_i = np.where(pos_cnt > 0, -pos_sum / np.maximum(pos_cnt, 1.0) + lse, 0.0)
    cls_sum = np.bincount(op_ids, weights=loss_i, minlength=n_op_i)
    cls_loss = np.where(counts > 0, cls_sum / np.maximum(counts, 1.0), 0.0)
    return np.float32(cls_loss.mean())



# revision 8
# speedup vs baseline: 1.4491x; 1.4491x over previous
"""Supervised-contrastive loss on 8 Trainium2 NeuronCores.

Math (reference):
    z = x / max(||x||, 1e-8)                  row-normalize
    sim = (z @ z.T) / TEMP                    [N, N]
    per-anchor: pos-mean over same-class (excl. self) and logsumexp over
    j != i, then per-class mean, then mean over classes.

Distribution — symmetric-block scheme.  The 8192 anchors are split into
16 slabs of 512; core c owns slabs 2c and 2c+1 (rows).  Each slab sigma
computes its sim block against col-slabs sigma..sigma+8 (mod 16):
  - distance 0 (diag) once, distances 1..7 once each (the transposed
    block's contribution is recovered from COLUMN sums), distance 8
    computed from both sides (so no colsum needed for it).
Per anchor both reductions are assembled on the host:
  esp[i] = sum_j exp(10*sim[i,j])   (row sums + column sums, self term
           subtracted exactly on host)
  tm[i,c] = z_i . W_c               (class-segment sums via small GEMM)
This computes each off-diagonal sim value once instead of twice:
216 fp8-DoubleRow matmuls per core instead of 384.

Per (slab s, m-chunk): three [128, 1536] PSUM tiles; ScalarE exps each
tile into an SBUF staging tile (row sums ride on accum_out); the DVE
accumulates the distance-1..7 portions into a per-slab [128, 3584] fp32
accumulator; at slab end the accumulator is cast to bf16 and a ones
[128,32]-stationary matmul reduces the 128 partitions (chunk k lands on
PSUM partitions 32j of two [128,512] tiles), copied out and DMA'd.

Hardware pitfalls baked in (from the previous kernel's iterations):
DMAs only from nc.sync, one matmul accumulation group per PSUM bank,
full-128-partition DoubleRow outputs only, one EXP per psum tile.
"""

import numpy as np
import ml_dtypes

N = 8192          # anchors
D = 768           # feature dim
NOP = 64          # number of classes
CORES = 8
NSLAB = 16        # row slabs
SW = 512          # slab width
NCHUNK = 10       # col chunks held per core (slabs 2c .. 2c+9)
LCOLS = NCHUNK * SW
KT8 = D // 256    # 3 double-row contraction tiles
TW = 1536         # main psum tile width (3 banks)
MT = 1024 // 128  # 8 m-chunks per core for tm (over both own slabs)
ACCW = 7 * SW     # per-slab colsum accumulator width
TEMP_INV = 10.0
EPS = 1e-8

FP8 = ml_dtypes.float8_e4m3

_CACHE = {}
LAST_RESULT = None  # BassKernelResults of the most recent run (for profiling)


def _build_nc():
    from concourse import bacc
    import concourse.mybir as mybir
    import concourse.tile as tile

    f8 = mybir.dt.float8e4
    f32 = mybir.dt.float32
    bf16 = mybir.dt.bfloat16
    Exp = mybir.ActivationFunctionType.Exp
    DR = mybir.MatmulPerfMode.DoubleRow

    nc = bacc.Bacc(
        "TRN2", target_bir_lowering=False, debug=False, enable_asserts=False
    )
    z8 = nc.dram_tensor("z8", [128, NCHUNK, KT8, 2, SW], f8, kind="ExternalInput").ap()
    w8 = nc.dram_tensor("w8", [128, KT8, 2, NOP], f8, kind="ExternalInput").ap()
    pout = nc.dram_tensor("pout", [128, 25], f32, kind="ExternalOutput").ap()
    tmo = nc.dram_tensor("tmo", [128, MT, NOP], f32, kind="ExternalOutput").ap()
    cso = nc.dram_tensor("cso", [4, 2, 2, SW], f32, kind="ExternalOutput").ap()

    # staging slice -> per-slab accumulator placement, per tile index t:
    # tile t covers local chunks {s+3t .. s+3t+2}; the accumulator covers
    # local chunks s+1..s+7 (dists 1..7).
    ACC_MAP = {0: (SW, TW, 0), 1: (0, TW, 1024), 2: (0, 1024, 2560)}

    with tile.TileContext(nc) as tc:
        with (
            tc.tile_pool(name="singles", bufs=1) as singles,
            tc.tile_pool(name="stgp", bufs=2) as stgp,
        ):
            w8_sb = singles.tile([128, KT8, 2, NOP], f8)
            nc.sync.dma_start(out=w8_sb, in_=w8)
            zt = []
            for c in range(NCHUNK):
                zc = singles.tile(
                    [128, KT8, 2, SW], f8, name=f"zt{c}", tag=f"zt{c}"
                )
                if c == 0:
                    # halves: the first tm matmuls only need cols [0:256]
                    nc.sync.dma_start(
                        out=zc[:, :, :, 0:256], in_=z8[:, c, :, :, 0:256]
                    )
                    nc.sync.dma_start(
                        out=zc[:, :, :, 256:SW], in_=z8[:, c, :, :, 256:SW]
                    )
                else:
                    nc.sync.dma_start(out=zc, in_=z8[:, c])
                zt.append(zc)

            pacc = singles.tile([128, 25], f32)
            tm_sb = singles.tile([128, MT, NOP], f32)
            acc = singles.tile([128, 2, ACCW], bf16)
            cs_sb = singles.tile([128, 2, 2, SW], f32)
            ones_bf = singles.tile([128, 32], bf16)
            nc.vector.memset(ones_bf, 1.0)
            nc.vector.memset(cs_sb, 0.0)

            ps = tc.alloc_tile_pool(name="ps", bufs=2, space="PSUM")

            # ---- class-segment sums: tm[:, m, c] = A_m @ W.T ----
            for m in range(MT):
                pst = ps.tile([128, NOP], f32, name="red_t", tag="red", bufs=2)
                ch, off = divmod(m * 128, SW)
                for kk in range(KT8):
                    nc.tensor.matmul(
                        pst,
                        zt[ch][:, kk, :, off:off + 128],
                        w8_sb[:, kk, :, :],
                        start=(kk == 0),
                        stop=(kk == KT8 - 1),
                        perf_mode=DR,
                    )
                nc.vector.tensor_copy(tm_sb[:, m, :], pst)
            nc.sync.dma_start(out=tmo, in_=tm_sb)

            # ---- main slab sweep ----
            def colsum_reduce(s, g):
                """ones-matmul partition reduction of acc chunks g*4..; the
                distance-1..4 chunks (g=0) are final after the t=1 sweep,
                5..7 (g=1, incl. chunk 4) after t=2."""
                nmm = 4 if g == 0 else 3
                red = ps.tile([128, SW], f32, name="red_t", tag="red", bufs=2)
                for j in range(nmm):
                    k = g * 4 + j
                    nc.tensor.matmul(
                        red[32 * j:32 * (j + 1), :],
                        ones_bf,
                        acc[:, s, k * SW:(k + 1) * SW],
                        start=True,
                        stop=True,
                        tile_position=(0, 32 * j),
                    )
                nc.vector.tensor_copy(
                    cs_sb[0:32 * nmm, s, g, :], red[0:32 * nmm, :]
                )

            for t in range(3):
                for s in range(2):
                    for m in range(4):
                        last = t == 2 and s == 1 and m == 3
                        slot = (t * 2 + s) * 4 + m
                        if not last:
                            ps_t = ps.tile(
                                [128, TW], f32, name="mm_t", tag="mm", bufs=2
                            )
                            ps_parts = [(ps_t, 0, 3, slot)]
                        else:
                            # split the final tile so its colsum part (first
                            # 1024 cols) exps before the dup part finishes
                            ps_a = ps.tile(
                                [128, 1024], f32, name="mm_t", tag="mm", bufs=2
                            )
                            ps_b = ps.tile(
                                [128, SW], f32, name="mm_t", tag="mm", bufs=2
                            )
                            ps_parts = [(ps_a, 0, 2, slot), (ps_b, 2, 3, 24)]
                        for kk in range(KT8):
                            lhsT = zt[s][:, kk, :, m * 128:(m + 1) * 128]
                            for pst, j0, j1, _ in ps_parts:
                                for jj in range(j0, j1):
                                    c = s + 3 * t + jj
                                    nc.tensor.matmul(
                                        pst[:, (jj - j0) * SW:(jj - j0 + 1) * SW],
                                        lhsT,
                                        zt[c][:, kk, :, :],
                                        start=(kk == 0),
                                        stop=(kk == KT8 - 1),
                                        perf_mode=DR,
                                    )
                        lo, hi, aoff = ACC_MAP[t]
                        stg = stgp.tile(
                            [128, hi if last else TW], bf16,
                            name="stg_t", tag="stg",
                        )
                        for pst, j0, j1, pslot in ps_parts:
                            w = (j1 - j0) * SW
                            nc.scalar.activation(
                                out=stg[:, j0 * SW:j0 * SW + w] if not last
                                else (stg if pslot != 24 else pst),
                                in_=pst,
                                func=Exp,
                                scale=TEMP_INV,
                                accum_out=pacc[:, pslot:pslot + 1],
                            )
                        src = stg[:, lo:hi]
                        dst = acc[:, s, aoff:aoff + (hi - lo)]
                        if m == 0:
                            nc.vector.tensor_copy(dst, src)
                        else:
                            nc.vector.tensor_add(dst, dst, src)
                    if t >= 1:
                        colsum_reduce(s, t - 1)
            ps.release()

            nc.sync.dma_start(out=cso, in_=cs_sb[0:97:32, :, :, :])
            nc.sync.dma_start(out=pout, in_=pacc)

    nc.compile()
    return nc


def _get_nc():
    if "nc" not in _CACHE:
        _CACHE["nc"] = _build_nc()
    return _CACHE["nc"]


def _pack_dr(mat_t):
    """[D, cols] -> [128, KT8, 2, cols] with d = kk*256 + i*128 + p."""
    d, cols = mat_t.shape
    return np.ascontiguousarray(
        mat_t.reshape(KT8, 2, 128, cols).transpose(2, 0, 1, 3)
    )


def kernel(x, op_ids, n_op):
    global LAST_RESULT
    from concourse.bass_utils import run_bass_kernel_spmd

    x = np.asarray(x, dtype=np.float32).reshape(-1, D)
    op_ids = np.asarray(op_ids).reshape(-1).astype(np.int64)
    n_op_i = int(np.asarray(n_op))

    # ---- host prep: normalize, quantize, class sums, diagonal ----
    norms = np.sqrt((x.astype(np.float64) ** 2).sum(axis=1))
    norms = np.maximum(norms, EPS).astype(np.float32)
    z = x / norms[:, None]

    z8 = z.astype(FP8)
    z8f = z8.astype(np.float32)

    onehot = np.zeros((N, NOP), np.float32)
    onehot[np.arange(N), op_ids] = 1.0
    W8 = (onehot.T @ z8f).astype(FP8)               # [NOP, D] fp8

    z8_packed = _pack_dr(np.ascontiguousarray(z8.T))          # [128,3,2,N]
    w8_packed = _pack_dr(np.ascontiguousarray(W8.T.astype(FP8)))
    ssq = (z8f.astype(np.float64) ** 2).sum(axis=1)  # = sim[i, i]

    in_maps = []
    for c in range(CORES):
        zloc = np.stack(
            [
                z8_packed[:, :, :, ((2 * c + t) % NSLAB) * SW:
                          (((2 * c + t) % NSLAB) + 1) * SW]
                for t in range(NCHUNK)
            ],
            axis=1,
        )  # [128, 10, 3, 2, 512]
        in_maps.append(
            {"z8": np.ascontiguousarray(zloc), "w8": w8_packed}
        )

    nc = _get_nc()
    res = run_bass_kernel_spmd(nc, in_maps, core_ids=list(range(CORES)))
    LAST_RESULT = res

    # ---- host post: assemble esp from row + col sums, finish loss ----
    es = np.zeros(N, np.float64)
    tm_slabs = []
    for c in range(CORES):
        pout_c = res.results[c]["pout"].astype(np.float64)   # [128, 25]
        cso_c = res.results[c]["cso"].astype(np.float64)     # [4, 2, 2, 512]
        for s in range(2):
            sigma = (2 * c + s) % NSLAB
            for m in range(4):
                rows = sigma * SW + m * 128 + np.arange(128)
                es[rows] += sum(
                    pout_c[:, (t * 2 + s) * 4 + m] for t in range(3)
                )
                if s == 1 and m == 3:
                    # the split final tile wrote its dup-chunk row sums
                    # to the extra slot
                    es[rows] += pout_c[:, 24]
            for k in range(7):
                g, j = divmod(k, 4)
                tgt = ((sigma + 1 + k) % NSLAB) * SW
                es[tgt:tgt + SW] += cso_c[j, s, g, :]
        tm_slabs.append(
            res.results[c]["tmo"].transpose(1, 0, 2).reshape(1024, NOP)
        )
    tm_full = np.concatenate(tm_slabs).astype(np.float64)

    lse = np.log(es - np.exp(TEMP_INV * ssq))
    pos_sum = TEMP_INV * (tm_full[np.arange(N), op_ids] - ssq)
    counts = np.bincount(op_ids, minlength=n_op_i).astype(np.float64)
    pos_cnt = counts[op_ids] - 1.0

    loss_i = np.where(pos_cnt > 0, -pos_sum / np.maximum(pos_cnt, 1.0) + lse, 0.0)
    cls_sum = np.bincount(op_ids, weights=loss_i, minlength=n_op_i)
    cls_loss = np.where(counts > 0, cls_sum / np.maximum(counts, 1.0), 0.0)
    return np.float32(cls_loss.mean())


# revision 9
# speedup vs baseline: 1.4900x; 1.0282x over previous
"""Supervised-contrastive loss on 8 Trainium2 NeuronCores.

Math (reference):
    z = x / max(||x||, 1e-8)                  row-normalize
    sim = (z @ z.T) / TEMP                    [N, N]
    per-anchor: pos-mean over same-class (excl. self) and logsumexp over
    j != i, then per-class mean, then mean over classes.

Distribution — symmetric-block scheme.  The 8192 anchors are split into
16 slabs of 512; core c owns slabs 2c and 2c+1 (rows).  Each slab sigma
computes its sim block against col-slabs sigma..sigma+8 (mod 16):
  - distance 0 (diag) once, distances 1..7 once each (the transposed
    block's contribution is recovered from COLUMN sums), distance 8
    computed from both sides (so no colsum needed for it).
Per anchor both reductions are assembled on the host:
  esp[i] = sum_j exp(10*sim[i,j])   (row sums + column sums, self term
           subtracted exactly on host)
  tm[i,c] = z_i . W_c               (class-segment sums via small GEMM)
This computes each off-diagonal sim value once instead of twice:
216 fp8-DoubleRow matmuls per core instead of 384.

Per (slab s, m-chunk): three [128, 1536] PSUM tiles; ScalarE exps each
tile into an SBUF staging tile (row sums ride on accum_out); the DVE
accumulates the distance-1..7 portions into a per-slab [128, 3584] fp32
accumulator; at slab end the accumulator is cast to bf16 and a ones
[128,32]-stationary matmul reduces the 128 partitions (chunk k lands on
PSUM partitions 32j of two [128,512] tiles), copied out and DMA'd.

Hardware pitfalls baked in (from the previous kernel's iterations):
DMAs only from nc.sync, one matmul accumulation group per PSUM bank,
full-128-partition DoubleRow outputs only, one EXP per psum tile.
"""

import numpy as np
import ml_dtypes

N = 8192          # anchors
D = 768           # feature dim
NOP = 64          # number of classes
CORES = 8
NSLAB = 16        # row slabs
SW = 512          # slab width
NCHUNK = 10       # col chunks held per core (slabs 2c .. 2c+9)
LCOLS = NCHUNK * SW
KT8 = D // 256    # 3 double-row contraction tiles
TW = 1536         # main psum tile width (3 banks)
MT = 1024 // 128  # 8 m-chunks per core for tm (over both own slabs)
ACCW = 7 * SW     # per-slab colsum accumulator width
TEMP_INV = 10.0
EPS = 1e-8

FP8 = ml_dtypes.float8_e4m3

_CACHE = {}
LAST_RESULT = None  # BassKernelResults of the most recent run (for profiling)


def _build_nc():
    from concourse import bacc
    import concourse.mybir as mybir
    import concourse.tile as tile

    f8 = mybir.dt.float8e4
    f32 = mybir.dt.float32
    bf16 = mybir.dt.bfloat16
    Exp = mybir.ActivationFunctionType.Exp
    DR = mybir.MatmulPerfMode.DoubleRow

    nc = bacc.Bacc(
        "TRN2", target_bir_lowering=False, debug=False, enable_asserts=False
    )
    z8 = nc.dram_tensor("z8", [128, NCHUNK, KT8, 2, SW], f8, kind="ExternalInput").ap()
    w8 = nc.dram_tensor("w8", [128, KT8, 2, NOP], f8, kind="ExternalInput").ap()
    pout = nc.dram_tensor("pout", [128, 25], f32, kind="ExternalOutput").ap()
    tmo = nc.dram_tensor("tmo", [128, MT, NOP], f32, kind="ExternalOutput").ap()
    cso = nc.dram_tensor("cso", [4, 2, 2, SW], f32, kind="ExternalOutput").ap()

    # staging slice -> per-slab accumulator placement, per tile index t:
    # tile t covers local chunks {s+3t .. s+3t+2}; the accumulator covers
    # local chunks s+1..s+7 (dists 1..7).
    ACC_MAP = {0: (SW, TW, 0), 1: (0, TW, 1024), 2: (0, 1024, 2560)}

    with tile.TileContext(nc) as tc:
        with (
            tc.tile_pool(name="singles", bufs=1) as singles,
            tc.tile_pool(name="stgp", bufs=2) as stgp,
        ):
            w8_sb = singles.tile([128, KT8, 2, NOP], f8)
            nc.sync.dma_start(out=w8_sb, in_=w8)
            zt = []
            for c in range(NCHUNK):
                zc = singles.tile(
                    [128, KT8, 2, SW], f8, name=f"zt{c}", tag=f"zt{c}"
                )
                if c == 0:
                    # halves: the first tm matmuls only need cols [0:256]
                    nc.sync.dma_start(
                        out=zc[:, :, :, 0:256], in_=z8[:, c, :, :, 0:256]
                    )
                    nc.sync.dma_start(
                        out=zc[:, :, :, 256:SW], in_=z8[:, c, :, :, 256:SW]
                    )
                else:
                    nc.sync.dma_start(out=zc, in_=z8[:, c])
                zt.append(zc)

            pacc = singles.tile([128, 25], f32)
            tm_sb = singles.tile([128, MT, NOP], f32)
            acc = singles.tile([128, 2, ACCW], bf16)
            cs_sb = singles.tile([128, 2, 2, SW], f32)
            ones_bf = singles.tile([128, 32], bf16)
            warm_sb = singles.tile([128, 640], f8)
            nc.vector.memset(warm_sb, 0.0)
            nc.vector.memset(ones_bf, 1.0)
            nc.vector.memset(cs_sb, 0.0)

            ps = tc.alloc_tile_pool(name="ps", bufs=2, space="PSUM")

            # ---- HAM warm-up: ~3.6us of data-independent matmuls so the
            # PE clock gate opens before the first DMA-gated real work ----
            warm_ps = ps.tile([128, SW], f32, name="red_t", tag="red", bufs=2)
            for i in range(16):
                nc.tensor.matmul(
                    warm_ps,
                    warm_sb[:, 0:128],
                    warm_sb[:, 128:640],
                    start=(i == 0),
                    stop=(i == 15),
                )
            nc.vector.tensor_copy(cs_sb[:, 0, 0, :], warm_ps)

            # ---- class-segment sums: tm[:, m, c] = A_m @ W.T ----
            for m in range(MT):
                pst = ps.tile([128, NOP], f32, name="red_t", tag="red", bufs=2)
                ch, off = divmod(m * 128, SW)
                for kk in range(KT8):
                    nc.tensor.matmul(
                        pst,
                        zt[ch][:, kk, :, off:off + 128],
                        w8_sb[:, kk, :, :],
                        start=(kk == 0),
                        stop=(kk == KT8 - 1),
                        perf_mode=DR,
                    )
                nc.vector.tensor_copy(tm_sb[:, m, :], pst)
            nc.sync.dma_start(out=tmo, in_=tm_sb)

            # ---- main slab sweep ----
            def colsum_reduce(s, g):
                """ones-matmul partition reduction of acc chunks g*4..; the
                distance-1..4 chunks (g=0) are final after the t=1 sweep,
                5..7 (g=1, incl. chunk 4) after t=2."""
                nmm = 4 if g == 0 else 3
                red = ps.tile([128, SW], f32, name="red_t", tag="red", bufs=2)
                for j in range(nmm):
                    k = g * 4 + j
                    nc.tensor.matmul(
                        red[32 * j:32 * (j + 1), :],
                        ones_bf,
                        acc[:, s, k * SW:(k + 1) * SW],
                        start=True,
                        stop=True,
                        tile_position=(0, 32 * j),
                    )
                nc.vector.tensor_copy(
                    cs_sb[0:32 * nmm, s, g, :], red[0:32 * nmm, :]
                )

            for t in range(3):
                for s in range(2):
                    for m in range(4):
                        last = t == 2 and s == 1 and m == 3
                        slot = (t * 2 + s) * 4 + m
                        if not last:
                            ps_t = ps.tile(
                                [128, TW], f32, name="mm_t", tag="mm", bufs=2
                            )
                            ps_parts = [(ps_t, 0, 3, slot)]
                        else:
                            # split the final tile so its colsum part (first
                            # 1024 cols) exps before the dup part finishes
                            ps_a = ps.tile(
                                [128, 1024], f32, name="mm_t", tag="mm", bufs=2
                            )
                            ps_b = ps.tile(
                                [128, SW], f32, name="mm_t", tag="mm", bufs=2
                            )
                            ps_parts = [(ps_a, 0, 2, slot), (ps_b, 2, 3, 24)]
                        for kk in range(KT8):
                            lhsT = zt[s][:, kk, :, m * 128:(m + 1) * 128]
                            for pst, j0, j1, _ in ps_parts:
                                for jj in range(j0, j1):
                                    c = s + 3 * t + jj
                                    nc.tensor.matmul(
                                        pst[:, (jj - j0) * SW:(jj - j0 + 1) * SW],
                                        lhsT,
                                        zt[c][:, kk, :, :],
                                        start=(kk == 0),
                                        stop=(kk == KT8 - 1),
                                        perf_mode=DR,
                                    )
                        lo, hi, aoff = ACC_MAP[t]
                        stg = stgp.tile(
                            [128, hi if last else TW], bf16,
                            name="stg_t", tag="stg",
                        )
                        for pst, j0, j1, pslot in ps_parts:
                            w = (j1 - j0) * SW
                            nc.scalar.activation(
                                out=stg[:, j0 * SW:j0 * SW + w] if not last
                                else (stg if pslot != 24 else pst),
                                in_=pst,
                                func=Exp,
                                scale=TEMP_INV,
                                accum_out=pacc[:, pslot:pslot + 1],
                            )
                        src = stg[:, lo:hi]
                        dst = acc[:, s, aoff:aoff + (hi - lo)]
                        if m == 0:
                            nc.vector.tensor_copy(dst, src)
                        else:
                            nc.vector.tensor_add(dst, dst, src)
                    if t >= 1:
                        colsum_reduce(s, t - 1)
            ps.release()

            nc.sync.dma_start(out=cso, in_=cs_sb[0:97:32, :, :, :])
            nc.sync.dma_start(out=pout, in_=pacc)

    nc.compile()
    return nc


def _get_nc():
    if "nc" not in _CACHE:
        _CACHE["nc"] = _build_nc()
    return _CACHE["nc"]


def _pack_dr(mat_t):
    """[D, cols] -> [128, KT8, 2, cols] with d = kk*256 + i*128 + p."""
    d, cols = mat_t.shape
    return np.ascontiguousarray(
        mat_t.reshape(KT8, 2, 128, cols).transpose(2, 0, 1, 3)
    )


def kernel(x, op_ids, n_op):
    global LAST_RESULT
    from concourse.bass_utils import run_bass_kernel_spmd

    x = np.asarray(x, dtype=np.float32).reshape(-1, D)
    op_ids = np.asarray(op_ids).reshape(-1).astype(np.int64)
    n_op_i = int(np.asarray(n_op))

    # ---- host prep: normalize, quantize, class sums, diagonal ----
    norms = np.sqrt((x.astype(np.float64) ** 2).sum(axis=1))
    norms = np.maximum(norms, EPS).astype(np.float32)
    z = x / norms[:, None]

    z8 = z.astype(FP8)
    z8f = z8.astype(np.float32)

    onehot = np.zeros((N, NOP), np.float32)
    onehot[np.arange(N), op_ids] = 1.0
    W8 = (onehot.T @ z8f).astype(FP8)               # [NOP, D] fp8

    z8_packed = _pack_dr(np.ascontiguousarray(z8.T))          # [128,3,2,N]
    w8_packed = _pack_dr(np.ascontiguousarray(W8.T.astype(FP8)))
    ssq = (z8f.astype(np.float64) ** 2).sum(axis=1)  # = sim[i, i]

    in_maps = []
    for c in range(CORES):
        zloc = np.stack(
            [
                z8_packed[:, :, :, ((2 * c + t) % NSLAB) * SW:
                          (((2 * c + t) % NSLAB) + 1) * SW]
                for t in range(NCHUNK)
            ],
            axis=1,
        )  # [128, 10, 3, 2, 512]
        in_maps.append(
            {"z8": np.ascontiguousarray(zloc), "w8": w8_packed}
        )

    nc = _get_nc()
    res = run_bass_kernel_spmd(nc, in_maps, core_ids=list(range(CORES)))
    LAST_RESULT = res

    # ---- host post: assemble esp from row + col sums, finish loss ----
    es = np.zeros(N, np.float64)
    tm_slabs = []
    for c in range(CORES):
        pout_c = res.results[c]["pout"].astype(np.float64)   # [128, 25]
        cso_c = res.results[c]["cso"].astype(np.float64)     # [4, 2, 2, 512]
        for s in range(2):
            sigma = (2 * c + s) % NSLAB
            for m in range(4):
                rows = sigma * SW + m * 128 + np.arange(128)
                es[rows] += sum(
                    pout_c[:, (t * 2 + s) * 4 + m] for t in range(3)
                )
                if s == 1 and m == 3:
                    # the split final tile wrote its dup-chunk row sums
                    # to the extra slot
                    es[rows] += pout_c[:, 24]
            for k in range(7):
                g, j = divmod(k, 4)
                tgt = ((sigma + 1 + k) % NSLAB) * SW
                es[tgt:tgt + SW] += cso_c[j, s, g, :]
        tm_slabs.append(
            res.results[c]["tmo"].transpose(1, 0, 2).reshape(1024, NOP)
        )
    tm_full = np.concatenate(tm_slabs).astype(np.float64)

    lse = np.log(es - np.exp(TEMP_INV * ssq))
    pos_sum = TEMP_INV * (tm_full[np.arange(N), op_ids] - ssq)
    counts = np.bincount(op_ids, minlength=n_op_i).astype(np.float64)
    pos_cnt = counts[op_ids] - 1.0

    loss_i = np.where(pos_cnt > 0, -pos_sum / np.maximum(pos_cnt, 1.0) + lse, 0.0)
    cls_sum = np.bincount(op_ids, weights=loss_i, minlength=n_op_i)
    cls_loss = np.where(counts > 0, cls_sum / np.maximum(counts, 1.0), 0.0)
    return np.float32(cls_loss.mean())
